# revision 12
# baseline (speedup 1.0000x reference)
"""Non-Local Means (gray-weighted) Bass kernel for Trainium2.

Contract: kernel(rgb, h) with rgb [8,3,512,512] f32, h [1] f32 -> [8,3,512,512] f32.
Data-parallel over batch: one image per NeuronCore (8 cores).

Algorithm (matches reference.py):
  y = luminance(clip(rgb,0,1)); for each shift s in [-R,R]^2:
    dist_s = sqrt(relu(box7((y - roll(y,s))^2)))   (circular boundary)
    w_s = exp(-dist_s/(relu(h)+eps))
    num += roll(rgb,s)*w_s ; den += w_s
  out = clip(num/den, 0, 1)

Mapping per core:
  - Symmetric pairs: dist_{-s}(p) = dist_s(p+s). Each pair (s,-s) computes one
    dist plane; the -s side uses dist read at +s (rows shifted on the
    TensorEngine via shifted-identity matmuls, x shifted via haloed AP reads)
    and a second exp.
  - Row-shifted copies of the [y,rgb] block are grouped by dy (one +dy and one
    -dy block per group) via SBUF->SBUF DMA; dx handled by x-offset reads
    against +-R x-halos.
  - 7x7 box = two banded-circulant matmul stages on the TensorEngine with the
    image data as the stationary operand; each stage transposes, so two
    stages land back in the original layout.
  - sqrt/exp on ScalarE; elementwise + accumulation on VectorE.
"""

import sys

sys.path.insert(0, "/opt/trn_rl_repo")

import numpy as np

EPS = 1e-8
PR = 3  # patch radius (7x7 box)
P = 128  # SBUF partitions

_CACHE = {}


def _runs_mod(start, length, m):
    """Split indices [(start+j) % m for j in range(length)] into contiguous
    runs; yields (out_start, window_offset, run_len)."""
    out = []
    j = 0
    while j < length:
        g = (start + j) % m
        run = min(length - j, m - g)
        out.append((g, j, run))
        j += run
    return out


def _build(H, W, R, n_cores):
    import concourse.bacc as bacc
    import concourse.mybir as mybir
    import concourse.tile as tile
    from concourse.mybir import ActivationFunctionType as AF
    from concourse.mybir import AluOpType as Op

    F32 = mybir.dt.float32
    C = H // P  # row chunks
    XB = W // P  # x blocks
    WB = W + 2 * R  # x-haloed width
    BW = P + 2 * PR  # band window width

    nc = bacc.Bacc(None, target_bir_lowering=False, debug=False)

    rgb_in = nc.dram_tensor("rgb", [3, H, W], F32, kind="ExternalInput")
    h_in = nc.dram_tensor("h", [1, 1], F32, kind="ExternalInput")
    band_in = nc.dram_tensor("band", [P, BW], F32, kind="ExternalInput")
    id_in = nc.dram_tensor("idents", [P, 2 * R * P], F32, kind="ExternalInput")
    out_dram = nc.dram_tensor("out", [3, H, W], F32, kind="ExternalOutput")

    with tile.TileContext(nc) as tc:
        with (
            tc.tile_pool(name="res", bufs=1) as res,
            tc.tile_pool(name="roll", bufs=1) as rollp,
            tc.tile_pool(name="work", bufs=1) as work,
            tc.tile_pool(name="psA", bufs=2, space="PSUM") as psA,
            tc.tile_pool(name="psB", bufs=2, space="PSUM") as psB,
            tc.tile_pool(name="psD", bufs=2, space="PSUM") as psD,
        ):
            # ---------------- setup ----------------
            yrgb = res.tile([P, C, 4, WB], F32)  # plane 0=y, 1..3=rgb
            acc = res.tile([P, C, 3, W], F32)
            den = res.tile([P, C, W], F32)
            band = res.tile([P, BW], F32)
            idents = res.tile([P, 2 * R * P], F32)
            h_sb = res.tile([1, 1], F32)
            nih1 = res.tile([1, 1], F32)
            nih = res.tile([P, 1], F32)  # -1/(relu(h)+eps) on all partitions

            nc.sync.dma_start(band[:, :], band_in[:, :])
            nc.sync.dma_start(idents[:, :], id_in[:, :])
            nc.sync.dma_start(h_sb[:, :], h_in[:, :])
            nc.scalar.activation(h_sb[:, :], h_sb[:, :], AF.Relu)
            nc.vector.tensor_scalar_add(h_sb[:, :], h_sb[:, :], EPS)
            nc.vector.reciprocal(nih1[:, :], h_sb[:, :])
            nc.vector.tensor_scalar_mul(nih1[:, :], nih1[:, :], -1.0)
            nc.gpsimd.partition_broadcast(nih[:, :], nih1[:, :])

            for ch in range(3):
                rgb_src = rgb_in.ap()[ch].rearrange("(c p) x -> p c x", p=P)
                nc.sync.dma_start(yrgb[:, :, 1 + ch, R : R + W], rgb_src)
            nc.vector.tensor_scalar(
                yrgb[:, :, 1:4, R : R + W],
                yrgb[:, :, 1:4, R : R + W],
                0.0,
                1.0,
                Op.max,
                Op.min,
            )
            tmp0 = work.tile([P, C, W], F32, tag="d")
            yc = yrgb[:, :, 0, R : R + W]
            nc.vector.tensor_scalar_mul(yc, yrgb[:, :, 1, R : R + W], 0.299)
            nc.vector.tensor_scalar_mul(tmp0[:, :, :], yrgb[:, :, 2, R : R + W], 0.587)
            nc.vector.tensor_tensor(yc, yc, tmp0[:, :, :], Op.add)
            nc.vector.tensor_scalar_mul(tmp0[:, :, :], yrgb[:, :, 3, R : R + W], 0.114)
            nc.vector.tensor_tensor(yc, yc, tmp0[:, :, :], Op.add)
            nc.vector.tensor_copy(yrgb[:, :, :, 0:R], yrgb[:, :, :, W : W + R])
            nc.vector.tensor_copy(
                yrgb[:, :, :, W + R : W + 2 * R], yrgb[:, :, :, R : 2 * R]
            )

            # zero-shift term (w=1)
            nc.vector.tensor_copy(acc[:, :, :, :], yrgb[:, :, 1:4, R : R + W])
            nc.vector.memset(den[:, :, :], 1.0)

            def box_stage(pool, tag, src, n_chunks, n_blocks, m_total, dst):
                """dst[:, b, m] = sum_k band-circulant matmul of src chunks."""
                for b in range(n_blocks):
                    ps = pool.tile([P, m_total], F32, tag=tag, name=tag)
                    mms = []
                    for t in range(n_chunks):
                        for g, off, run in _runs_mod(P * t - PR, BW, m_total):
                            mms.append((t, g, off, run))
                    for i, (t, g, off, run) in enumerate(mms):
                        nc.tensor.matmul(
                            ps[:, g : g + run],
                            src[:, t, b * P : (b + 1) * P],
                            band[:, off : off + run],
                            start=(i == 0),
                            stop=(i == len(mms) - 1),
                        )
                    if dst is not None:
                        nc.scalar.copy(dst[:, b, :], ps[:, :])
                    else:
                        yield b, ps

            # ---------------- shifts, grouped by dy ----------------
            for dy in range(0, R + 1):
                dxs = list(range(-R, R + 1)) if dy > 0 else list(range(1, R + 1))
                if dy == 0:
                    ysP = ysM = yrgb
                else:
                    # ysP[r] = yrgb[r-dy] ; ysM[r] = yrgb[r+dy] (rows circular)
                    ysP = rollp.tile([P, C, 4, WB], F32, tag="ysP", name="ysP")
                    ysM = rollp.tile([P, C, 4, WB], F32, tag="ysM", name="ysM")
                    nc.sync.dma_start(ysP[dy:P, :, :, :], yrgb[0 : P - dy, :, :, :])
                    if C > 1:
                        nc.sync.dma_start(
                            ysP[0:dy, 1:C, :, :], yrgb[P - dy : P, 0 : C - 1, :, :]
                        )
                    nc.sync.dma_start(
                        ysP[0:dy, 0, :, :], yrgb[P - dy : P, C - 1, :, :]
                    )
                    nc.sync.dma_start(ysM[0 : P - dy, :, :, :], yrgb[dy:P, :, :, :])
                    if C > 1:
                        nc.sync.dma_start(
                            ysM[P - dy : P, 0 : C - 1, :, :], yrgb[0:dy, 1:C, :, :]
                        )
                    nc.sync.dma_start(
                        ysM[P - dy : P, C - 1, :, :], yrgb[0:dy, 0, :, :]
                    )
                if dy > 0:
                    i1 = idents[:, 2 * (dy - 1) * P : (2 * dy - 1) * P]
                    i2 = idents[:, (2 * dy - 1) * P : 2 * dy * P]

                for dx in dxs:
                    xf = slice(R - dx, R - dx + W)  # read at x - dx
                    xb = slice(R + dx, R + dx + W)  # read at x + dx

                    # D = (y - y(p-s))^2
                    dbuf = work.tile([P, C, W], F32, tag="d", name="dbuf")
                    nc.vector.tensor_tensor(
                        dbuf[:, :, :], yc, ysP[:, :, 0, xf], Op.subtract
                    )
                    nc.scalar.activation(dbuf[:, :, :], dbuf[:, :, :], AF.Square)

                    # box over rows (output transposed: [x, r])
                    t1s = work.tile([P, XB, H], F32, tag="t1s", name="t1s")
                    list(box_stage(psA, "t1ps", dbuf, C, XB, H, t1s))
                    # box over x (output back to [r, x]); dist = sqrt(B) straight
                    # from PSUM into x-haloed bm (B >= 0: all-positive sums)
                    bm = work.tile([P, C, WB], F32, tag="bm", name="bm")
                    for rb, ps in box_stage(psB, "bps", t1s, XB, C, W, None):
                        nc.scalar.activation(bm[:, rb, R : R + W], ps[:, :], AF.Sqrt)
                    bmc = bm[:, :, R : R + W]
                    # x halos of dist (for the backward x+dx read)
                    nc.scalar.copy(bm[:, :, 0:R], bm[:, :, W : W + R])
                    nc.scalar.copy(
                        bm[:, :, W + R : W + 2 * R], bm[:, :, R : 2 * R]
                    )

                    # w1 = exp(-dist/h); forward apply
                    w1 = work.tile([P, C, W], F32, tag="w1", name="w1")
                    nc.scalar.activation(w1[:, :, :], bmc, AF.Exp, scale=nih[:, :])
                    u = work.tile([P, C, W], F32, tag="u", name="u")
                    for ch in range(3):
                        nc.vector.tensor_tensor(
                            u[:, :, :], ysP[:, :, 1 + ch, xf], w1[:, :, :], Op.mult
                        )
                        nc.vector.tensor_tensor(
                            acc[:, :, ch, :], acc[:, :, ch, :], u[:, :, :], Op.add
                        )
                    nc.gpsimd.tensor_tensor(
                        den[:, :, :], den[:, :, :], w1[:, :, :], Op.add
                    )

                    # w2 = exp(-dist(p+s)/h)
                    w2 = work.tile([P, C, W], F32, tag="w2", name="w2")
                    if dy == 0:
                        nc.scalar.activation(
                            w2[:, :, :], bm[:, :, xb], AF.Exp, scale=nih[:, :]
                        )
                    else:
                        # rows shifted by +dy on the PE: out[m] = dist[m+dy]
                        pss = []
                        for c in range(C):
                            ps = psD.tile([P, W], F32, tag="d2", name="d2")
                            nc.tensor.matmul(
                                ps[:, :], i1, bm[:, c, xb], start=True, stop=False
                            )
                            pss.append(ps)
                        for c in range(C):
                            nc.tensor.matmul(
                                pss[c][:, :],
                                i2,
                                bm[:, (c + 1) % C, xb],
                                start=False,
                                stop=True,
                            )
                        for c in range(C):
                            nc.scalar.activation(
                                w2[:, c, :], pss[c][:, :], AF.Exp, scale=nih[:, :]
                            )

                    # backward apply: num += rgb(p+s)*w2 ; den += w2
                    for ch in range(3):
                        nc.vector.tensor_tensor(
                            u[:, :, :], ysM[:, :, 1 + ch, xb], w2[:, :, :], Op.mult
                        )
                        nc.vector.tensor_tensor(
                            acc[:, :, ch, :], acc[:, :, ch, :], u[:, :, :], Op.add
                        )
                    nc.gpsimd.tensor_tensor(
                        den[:, :, :], den[:, :, :], w2[:, :, :], Op.add
                    )

            # ---------------- output ----------------
            rden = work.tile([P, C, W], F32, tag="d", name="rden")
            nc.vector.reciprocal(rden[:, :, :], den[:, :, :])
            for ch in range(3):
                nc.vector.tensor_tensor(
                    acc[:, :, ch, :], acc[:, :, ch, :], rden[:, :, :], Op.mult
                )
            nc.vector.tensor_scalar(
                acc[:, :, :, :], acc[:, :, :, :], 0.0, 1.0, Op.max, Op.min
            )
            for ch in range(3):
                out_dst = out_dram.ap()[ch].rearrange("(c p) x -> p c x", p=P)
                nc.sync.dma_start(out_dst, acc[:, :, ch, :])

    nc.compile()
    return nc


def _band_matrix():
    bw = P + 2 * PR
    i = np.arange(P)[:, None]
    j = np.arange(bw)[None, :]
    return (((j - i) >= 0) & ((j - i) <= 2 * PR)).astype(np.float32)


def _ident_matrices(R):
    """Packed shifted identities [P, 2*R*P]: for dy in 1..R, I1 (out[m]=in[m+dy]
    within chunk) then I2 (wrap rows from the next chunk)."""
    out = np.zeros((P, 2 * R * P), np.float32)
    for dy in range(1, R + 1):
        i1 = np.zeros((P, P), np.float32)
        i2 = np.zeros((P, P), np.float32)
        for m in range(P - dy):
            i1[m + dy, m] = 1.0
        for m in range(P - dy, P):
            i2[m + dy - P, m] = 1.0
        out[:, 2 * (dy - 1) * P : (2 * dy - 1) * P] = i1
        out[:, (2 * dy - 1) * P : 2 * dy * P] = i2
    return out


def get_nc(H=512, W=512, R=10, n_cores=8):
    key = (H, W, R, n_cores)
    if key not in _CACHE:
        _CACHE[key] = _build(H, W, R, n_cores)
    return _CACHE[key]


def run(rgb, h, H, W, R):
    """rgb [B,3,H,W], h [1] -> [B,3,H,W]; B must equal n_cores used."""
    from concourse.bass_utils import run_bass_kernel_spmd

    B = rgb.shape[0]
    nc = get_nc(H, W, R, B)
    band = _band_matrix()
    idents = _ident_matrices(R)
    hv = np.asarray(h, np.float32).reshape(1, 1)
    in_maps = [
        {
            "rgb": np.ascontiguousarray(rgb[i], np.float32),
            "h": hv,
            "band": band,
            "idents": idents,
        }
        for i in range(B)
    ]
    res = run_bass_kernel_spmd(nc, in_maps, list(range(B)))
    return np.stack([res.results[i]["out"] for i in range(B)], axis=0)


def kernel(rgb, h):
    rgb = np.asarray(rgb, np.float32)
    out = run(rgb, np.asarray(h, np.float32), 512, 512, 10)
    return out.astype(np.float32)


# revision 24
# speedup vs baseline: 372.6519x; 372.6519x over previous
"""Non-Local Means (gray-weighted) Bass kernel for Trainium2.

Contract: kernel(rgb, h) with rgb [8,3,512,512] f32, h [1] f32 -> [8,3,512,512] f32.
Data-parallel over batch: one image per NeuronCore (8 cores).

Algorithm (matches reference.py):
  y = luminance(clip(rgb,0,1)); for each shift s in [-R,R]^2:
    dist_s = sqrt(relu(box7((y - roll(y,s))^2)))   (circular boundary)
    w_s = exp(-dist_s/(relu(h)+eps))
    num += roll(rgb,s)*w_s ; den += w_s
  out = clip(num/den, 0, 1)

Mapping per core:
  - Symmetric pairs: dist_{-s}(p) = dist_s(p+s). Each pair (s,-s) computes one
    dist plane; the -s side uses dist read at +s (rows shifted on the
    TensorEngine via shifted-identity matmuls, x shifted via haloed AP reads)
    and a second exp.
  - Row-shifted copies of the [y,rgb] block are grouped by dy (one +dy and one
    -dy block per group) via SBUF->SBUF DMA; dx handled by x-offset reads
    against +-R x-halos.
  - 7x7 box = two banded-circulant matmul stages on the TensorEngine with the
    image data as the stationary operand; each stage transposes, so two
    stages land back in the original layout.
  - sqrt/exp on ScalarE; elementwise + accumulation on VectorE.
"""

import sys

sys.path.insert(0, "/opt/trn_rl_repo")

import numpy as np

EPS = 1e-8
PR = 3  # patch radius (7x7 box)
P = 128  # SBUF partitions

_CACHE = {}


def _runs_mod(start, length, m):
    """Split indices [(start+j) % m for j in range(length)] into contiguous
    runs; yields (out_start, window_offset, run_len)."""
    out = []
    j = 0
    while j < length:
        g = (start + j) % m
        run = min(length - j, m - g)
        out.append((g, j, run))
        j += run
    return out


def _build(H, W, R, n_cores):
    import concourse.bacc as bacc
    import concourse.mybir as mybir
    import concourse.tile as tile
    from concourse.mybir import ActivationFunctionType as AF
    from concourse.mybir import AluOpType as Op

    F32 = mybir.dt.float32
    C = H // P  # row chunks
    XB = W // P  # x blocks
    WB = W + 2 * R  # x-haloed width
    BW = P + 2 * PR  # band window width

    nc = bacc.Bacc(None, target_bir_lowering=False, debug=False)

    rgb_in = nc.dram_tensor("rgb", [3, H, W], F32, kind="ExternalInput")
    h_in = nc.dram_tensor("h", [1, 1], F32, kind="ExternalInput")
    band_in = nc.dram_tensor("band", [P, BW], F32, kind="ExternalInput")
    id_in = nc.dram_tensor("idents", [P, (2 * R + 1) * P], F32, kind="ExternalInput")
    out_dram = nc.dram_tensor("out", [3, H, W], F32, kind="ExternalOutput")

    with tile.TileContext(nc) as tc:
        with (
            tc.tile_pool(name="res", bufs=1) as res,
            tc.tile_pool(name="roll", bufs=1) as rollp,
            tc.tile_pool(name="work", bufs=1) as work,
            tc.tile_pool(name="psA", bufs=1, space="PSUM") as psA,
            tc.tile_pool(name="psB", bufs=1, space="PSUM") as psB,
            tc.tile_pool(name="psD", bufs=2, space="PSUM") as psD,
            tc.tile_pool(name="psDen", bufs=1, space="PSUM") as psDen,
        ):
            # ---------------- setup ----------------
            yrgb = res.tile([P, C, 4, WB], F32)  # plane 0=y, 1..3=rgb
            acc = res.tile([P, C, 3, W], F32)
            denp = psDen.tile([P, C, W], F32)  # den - 1, accumulated on the PE
            band = res.tile([P, BW], F32)
            idents = res.tile([P, (2 * R + 1) * P], F32)
            h_sb = res.tile([1, 1], F32)
            nih1 = res.tile([1, 1], F32)
            nih = res.tile([P, 1], F32)  # -1/(relu(h)+eps) on all partitions

            nc.sync.dma_start(band[:, :], band_in[:, :])
            nc.sync.dma_start(idents[:, :], id_in[:, :])
            nc.sync.dma_start(h_sb[:, :], h_in[:, :])
            nc.scalar.activation(h_sb[:, :], h_sb[:, :], AF.Relu)
            nc.vector.tensor_scalar_add(h_sb[:, :], h_sb[:, :], EPS)
            nc.vector.reciprocal(nih1[:, :], h_sb[:, :])
            nc.vector.tensor_scalar_mul(nih1[:, :], nih1[:, :], -1.0)
            nc.gpsimd.partition_broadcast(nih[:, :], nih1[:, :])

            for ch in range(3):
                rgb_src = rgb_in.ap()[ch].rearrange("(c p) x -> p c x", p=P)
                nc.sync.dma_start(yrgb[:, :, 1 + ch, R : R + W], rgb_src)
            nc.vector.tensor_scalar(
                yrgb[:, :, 1:4, R : R + W],
                yrgb[:, :, 1:4, R : R + W],
                0.0,
                1.0,
                Op.max,
                Op.min,
            )
            tmp0 = work.tile([P, C, W], F32, tag="d")
            yc = yrgb[:, :, 0, R : R + W]
            nc.vector.tensor_scalar_mul(yc, yrgb[:, :, 1, R : R + W], 0.299)
            nc.vector.tensor_scalar_mul(tmp0[:, :, :], yrgb[:, :, 2, R : R + W], 0.587)
            nc.vector.tensor_tensor(yc, yc, tmp0[:, :, :], Op.add)
            nc.vector.tensor_scalar_mul(tmp0[:, :, :], yrgb[:, :, 3, R : R + W], 0.114)
            nc.vector.tensor_tensor(yc, yc, tmp0[:, :, :], Op.add)
            nc.vector.tensor_copy(yrgb[:, :, :, 0:R], yrgb[:, :, :, W : W + R])
            nc.vector.tensor_copy(
                yrgb[:, :, :, W + R : W + 2 * R], yrgb[:, :, :, R : 2 * R]
            )

            # zero-shift term (w=1); den's +1 is added at the output stage
            nc.vector.tensor_copy(acc[:, :, :, :], yrgb[:, :, 1:4, R : R + W])
            ident = idents[:, 2 * R * P : (2 * R + 1) * P]
            pairs_total = sum(
                len(range(-R, R + 1)) if dy > 0 else len(range(1, R + 1))
                for dy in range(0, R + 1)
            )
            # den matmuls run in 512-element (one PSUM bank) units so that
            # start=True (which clears a whole bank) is exactly the first
            # write of each bank, and same-bank writes are WAW-ordered.
            n_den_banks = (C * W) // 512
            den_mm = [0]
            den_total = 2 * pairs_total * n_den_banks

            def den_accum(w):
                dflat = denp[:, :, :].rearrange("p a b -> p (a b)")
                wflat = w[:, :, :].rearrange("p a b -> p (a b)")
                for b in range(n_den_banks):
                    nc.tensor.matmul(
                        dflat[:, b * 512 : (b + 1) * 512],
                        ident,
                        wflat[:, b * 512 : (b + 1) * 512],
                        start=(den_mm[0] < n_den_banks),
                        stop=(den_mm[0] >= den_total - n_den_banks),
                        skip_group_check=True,
                    )
                    den_mm[0] += 1

            def box_stage(pool, tag, src, n_chunks, n_blocks, m_total, dst):
                """dst[:, b, m] = sum_k band-circulant matmul of src chunks."""
                for b in range(n_blocks):
                    ps = pool.tile([P, m_total], F32, tag=tag, name=tag)
                    mms = []
                    for t in range(n_chunks):
                        for g, off, run in _runs_mod(P * t - PR, BW, m_total):
                            mms.append((t, g, off, run))
                    for i, (t, g, off, run) in enumerate(mms):
                        nc.tensor.matmul(
                            ps[:, g : g + run],
                            src[:, t, b * P : (b + 1) * P],
                            band[:, off : off + run],
                            start=(i == 0),
                            stop=(i == len(mms) - 1),
                        )
                    if dst is not None:
                        nc.scalar.copy(dst[:, b, :], ps[:, :])
                    else:
                        yield b, ps

            # ---------------- shifts, grouped by dy ----------------
            for dy in range(0, R + 1):
                dxs = list(range(-R, R + 1)) if dy > 0 else list(range(1, R + 1))
                if dy == 0:
                    ysP = ysM = yrgb
                else:
                    # ysP[r] = yrgb[r-dy] ; ysM[r] = yrgb[r+dy] (rows circular)
                    ysP = rollp.tile([P, C, 4, WB], F32, tag="ysP", name="ysP")
                    ysM = rollp.tile([P, C, 4, WB], F32, tag="ysM", name="ysM")
                    nc.sync.dma_start(ysP[dy:P, :, :, :], yrgb[0 : P - dy, :, :, :])
                    if C > 1:
                        nc.sync.dma_start(
                            ysP[0:dy, 1:C, :, :], yrgb[P - dy : P, 0 : C - 1, :, :]
                        )
                    nc.sync.dma_start(
                        ysP[0:dy, 0, :, :], yrgb[P - dy : P, C - 1, :, :]
                    )
                    nc.sync.dma_start(ysM[0 : P - dy, :, :, :], yrgb[dy:P, :, :, :])
                    if C > 1:
                        nc.sync.dma_start(
                            ysM[P - dy : P, 0 : C - 1, :, :], yrgb[0:dy, 1:C, :, :]
                        )
                    nc.sync.dma_start(
                        ysM[P - dy : P, C - 1, :, :], yrgb[0:dy, 0, :, :]
                    )
                if dy > 0:
                    i1 = idents[:, 2 * (dy - 1) * P : (2 * dy - 1) * P]
                    i2 = idents[:, (2 * dy - 1) * P : 2 * dy * P]

                for dx in dxs:
                    xf = slice(R - dx, R - dx + W)  # read at x - dx
                    xb = slice(R + dx, R + dx + W)  # read at x + dx

                    # D = (y - y(p-s))^2
                    dbuf = work.tile([P, C, W], F32, tag="d", name="dbuf")
                    nc.vector.tensor_tensor(
                        dbuf[:, :, :], yc, ysP[:, :, 0, xf], Op.subtract
                    )
                    nc.scalar.activation(dbuf[:, :, :], dbuf[:, :, :], AF.Square)

                    # box over rows (output transposed: [x, r])
                    t1s = work.tile([P, XB, H], F32, tag="t1s", name="t1s")
                    list(box_stage(psA, "t1ps", dbuf, C, XB, H, t1s))
                    # box over x (output back to [r, x]); dist = sqrt(B) straight
                    # from PSUM into x-haloed bm (B >= 0: all-positive sums)
                    bm = work.tile([P, C, WB], F32, tag="bm", name="bm", bufs=2)
                    for rb, ps in box_stage(psB, "bps", t1s, XB, C, W, None):
                        nc.scalar.activation(bm[:, rb, R : R + W], ps[:, :], AF.Sqrt)
                    bmc = bm[:, :, R : R + W]
                    # x halos of dist (for the backward x+dx read)
                    nc.scalar.copy(bm[:, :, 0:R], bm[:, :, W : W + R])
                    nc.scalar.copy(
                        bm[:, :, W + R : W + 2 * R], bm[:, :, R : 2 * R]
                    )

                    # w1 = exp(-dist/h); forward apply
                    w1 = work.tile([P, C, W], F32, tag="w1", name="w1", bufs=2)
                    nc.scalar.activation(w1[:, :, :], bmc, AF.Exp, scale=nih[:, :])
                    u = work.tile([P, C, W], F32, tag="u", name="u")
                    for ch in range(3):
                        nc.vector.tensor_tensor(
                            u[:, :, :], ysP[:, :, 1 + ch, xf], w1[:, :, :], Op.mult
                        )
                        nc.vector.tensor_tensor(
                            acc[:, :, ch, :], acc[:, :, ch, :], u[:, :, :], Op.add
                        )
                    den_accum(w1)

                    # w2 = exp(-dist(p+s)/h)
                    w2 = work.tile([P, C, W], F32, tag="w2", name="w2")
                    if dy == 0:
                        nc.scalar.activation(
                            w2[:, :, :], bm[:, :, xb], AF.Exp, scale=nih[:, :]
                        )
                    else:
                        # rows shifted by +dy on the PE: out[m] = dist[m+dy]
                        pss = []
                        for c in range(C):
                            ps = psD.tile([P, W], F32, tag="d2", name="d2")
                            nc.tensor.matmul(
                                ps[:, :], i1, bm[:, c, xb], start=True, stop=False
                            )
                            pss.append(ps)
                        for c in range(C):
                            nc.tensor.matmul(
                                pss[c][:, :],
                                i2,
                                bm[:, (c + 1) % C, xb],
                                start=False,
                                stop=True,
                            )
                        for c in range(C):
                            nc.scalar.activation(
                                w2[:, c, :], pss[c][:, :], AF.Exp, scale=nih[:, :]
                            )

                    # backward apply: num += rgb(p+s)*w2 ; den += w2
                    # (channel 2 on GPSIMD, channels 0,1 on DVE, den on PE)
                    for ch in range(2):
                        nc.vector.tensor_tensor(
                            u[:, :, :], ysM[:, :, 1 + ch, xb], w2[:, :, :], Op.mult
                        )
                        nc.vector.tensor_tensor(
                            acc[:, :, ch, :], acc[:, :, ch, :], u[:, :, :], Op.add
                        )
                    ug = work.tile([P, C, W], F32, tag="ug", name="ug")
                    nc.gpsimd.tensor_tensor(
                        ug[:, :, :], ysM[:, :, 3, xb], w2[:, :, :], Op.mult
                    )
                    nc.gpsimd.tensor_tensor(
                        acc[:, :, 2, :], acc[:, :, 2, :], ug[:, :, :], Op.add
                    )
                    den_accum(w2)

            # ---------------- output ----------------
            rden = work.tile([P, C, W], F32, tag="d", name="rden")
            # den = denp + 1 (zero-shift term)
            nc.vector.tensor_scalar_add(rden[:, :, :], denp[:, :, :], 1.0)
            nc.vector.reciprocal(rden[:, :, :], rden[:, :, :])
            for ch in range(3):
                nc.vector.tensor_tensor(
                    acc[:, :, ch, :], acc[:, :, ch, :], rden[:, :, :], Op.mult
                )
            nc.vector.tensor_scalar(
                acc[:, :, :, :], acc[:, :, :, :], 0.0, 1.0, Op.max, Op.min
            )
            for ch in range(3):
                out_dst = out_dram.ap()[ch].rearrange("(c p) x -> p c x", p=P)
                nc.sync.dma_start(out_dst, acc[:, :, ch, :])

    nc.compile()
    return nc


def _band_matrix():
    bw = P + 2 * PR
    i = np.arange(P)[:, None]
    j = np.arange(bw)[None, :]
    return (((j - i) >= 0) & ((j - i) <= 2 * PR)).astype(np.float32)


def _ident_matrices(R):
    """Packed shifted identities [P, (2*R+1)*P]: for dy in 1..R, I1
    (out[m]=in[m+dy] within chunk) then I2 (wrap rows from the next chunk);
    the last block is the plain identity."""
    out = np.zeros((P, (2 * R + 1) * P), np.float32)
    for dy in range(1, R + 1):
        i1 = np.zeros((P, P), np.float32)
        i2 = np.zeros((P, P), np.float32)
        for m in range(P - dy):
            i1[m + dy, m] = 1.0
        for m in range(P - dy, P):
            i2[m + dy - P, m] = 1.0
        out[:, 2 * (dy - 1) * P : (2 * dy - 1) * P] = i1
        out[:, (2 * dy - 1) * P : 2 * dy * P] = i2
    out[:, 2 * R * P : (2 * R + 1) * P] = np.eye(P, dtype=np.float32)
    return out


def get_nc(H=512, W=512, R=10, n_cores=8):
    key = (H, W, R, n_cores)
    if key not in _CACHE:
        _CACHE[key] = _build(H, W, R, n_cores)
    return _CACHE[key]


def run(rgb, h, H, W, R):
    """rgb [B,3,H,W], h [1] -> [B,3,H,W]; B must equal n_cores used."""
    from concourse.bass_utils import run_bass_kernel_spmd

    B = rgb.shape[0]
    nc = get_nc(H, W, R, B)
    band = _band_matrix()
    idents = _ident_matrices(R)
    hv = np.asarray(h, np.float32).reshape(1, 1)
    in_maps = [
        {
            "rgb": np.ascontiguousarray(rgb[i], np.float32),
            "h": hv,
            "band": band,
            "idents": idents,
        }
        for i in range(B)
    ]
    res = run_bass_kernel_spmd(nc, in_maps, list(range(B)))
    return np.stack([res.results[i]["out"] for i in range(B)], axis=0)


def kernel(rgb, h):
    rgb = np.asarray(rgb, np.float32)
    out = run(rgb, np.asarray(h, np.float32), 512, 512, 10)
    return out.astype(np.float32)


# revision 29
# speedup vs baseline: 404.3011x; 1.0849x over previous
"""Non-Local Means (gray-weighted) Bass kernel for Trainium2.

Contract: kernel(rgb, h) with rgb [8,3,512,512] f32, h [1] f32 -> [8,3,512,512] f32.
Data-parallel over batch: one image per NeuronCore (8 cores).

Algorithm (matches reference.py):
  y = luminance(clip(rgb,0,1)); for each shift s in [-R,R]^2:
    dist_s = sqrt(relu(box7((y - roll(y,s))^2)))   (circular boundary)
    w_s = exp(-dist_s/(relu(h)+eps))
    num += roll(rgb,s)*w_s ; den += w_s
  out = clip(num/den, 0, 1)

Mapping per core:
  - Symmetric pairs: dist_{-s}(p) = dist_s(p+s). Each pair (s,-s) computes one
    dist plane; the -s side uses dist read at +s (rows shifted on the
    TensorEngine via shifted-identity matmuls, x shifted via haloed AP reads)
    and a second exp.
  - Row-shifted copies of the [y,rgb] block are grouped by dy (one +dy and one
    -dy block per group) via SBUF->SBUF DMA; dx handled by x-offset reads
    against +-R x-halos.
  - 7x7 box = two banded-circulant matmul stages on the TensorEngine with the
    image data as the stationary operand; each stage transposes, so two
    stages land back in the original layout.
  - sqrt/exp on ScalarE; elementwise + accumulation on VectorE.
"""

import sys

sys.path.insert(0, "/opt/trn_rl_repo")

import numpy as np

EPS = 1e-8
PR = 3  # patch radius (7x7 box)
P = 128  # SBUF partitions

_CACHE = {}


def _runs_mod(start, length, m):
    """Split indices [(start+j) % m for j in range(length)] into contiguous
    runs; yields (out_start, window_offset, run_len)."""
    out = []
    j = 0
    while j < length:
        g = (start + j) % m
        run = min(length - j, m - g)
        out.append((g, j, run))
        j += run
    return out


def _build(H, W, R, n_cores):
    import concourse.bacc as bacc
    import concourse.mybir as mybir
    import concourse.tile as tile
    from concourse.mybir import ActivationFunctionType as AF
    from concourse.mybir import AluOpType as Op

    F32 = mybir.dt.float32
    C = H // P  # row chunks
    XB = W // P  # x blocks
    WB = W + 2 * R  # x-haloed width
    BW = P + 2 * PR  # band window width

    nc = bacc.Bacc(None, target_bir_lowering=False, debug=False)

    rgb_in = nc.dram_tensor("rgb", [3, H, W], F32, kind="ExternalInput")
    h_in = nc.dram_tensor("h", [1, 1], F32, kind="ExternalInput")
    band_in = nc.dram_tensor("band", [P, BW], F32, kind="ExternalInput")
    id_in = nc.dram_tensor("idents", [P, (2 * R + 1) * P], F32, kind="ExternalInput")
    out_dram = nc.dram_tensor("out", [3, H, W], F32, kind="ExternalOutput")

    with tile.TileContext(nc) as tc:
        with (
            tc.tile_pool(name="res", bufs=1) as res,
            tc.tile_pool(name="roll", bufs=1) as rollp,
            tc.tile_pool(name="work", bufs=1) as work,
            tc.tile_pool(name="psA", bufs=2, space="PSUM") as psA,
            tc.tile_pool(name="psB", bufs=1, space="PSUM") as psB,
            tc.tile_pool(name="psD", bufs=1, space="PSUM") as psD,
            tc.tile_pool(name="psDen", bufs=1, space="PSUM") as psDen,
        ):
            # ---------------- setup ----------------
            yrgb = res.tile([P, C, 4, WB], F32)  # plane 0=y, 1..3=rgb
            acc = res.tile([P, C, 3, W], F32)
            denp = psDen.tile([P, C, W], F32)  # den - 1, accumulated on the PE
            band = res.tile([P, BW], F32)
            identd = res.tile([P, P], F32)
            h_sb = res.tile([1, 1], F32)
            nih1 = res.tile([1, 1], F32)
            nih = res.tile([P, 1], F32)  # -1/(relu(h)+eps) on all partitions

            nc.sync.dma_start(band[:, :], band_in[:, :])
            nc.sync.dma_start(identd[:, :], id_in[:, 2 * R * P : (2 * R + 1) * P])
            nc.sync.dma_start(h_sb[:, :], h_in[:, :])
            nc.scalar.activation(h_sb[:, :], h_sb[:, :], AF.Relu)
            nc.vector.tensor_scalar_add(h_sb[:, :], h_sb[:, :], EPS)
            nc.vector.reciprocal(nih1[:, :], h_sb[:, :])
            nc.vector.tensor_scalar_mul(nih1[:, :], nih1[:, :], -1.0)
            nc.gpsimd.partition_broadcast(nih[:, :], nih1[:, :])

            for ch in range(3):
                rgb_src = rgb_in.ap()[ch].rearrange("(c p) x -> p c x", p=P)
                nc.sync.dma_start(yrgb[:, :, 1 + ch, R : R + W], rgb_src)
            nc.vector.tensor_scalar(
                yrgb[:, :, 1:4, R : R + W],
                yrgb[:, :, 1:4, R : R + W],
                0.0,
                1.0,
                Op.max,
                Op.min,
            )
            tmp0 = work.tile([P, C, W], F32, tag="d", bufs=2)
            yc = yrgb[:, :, 0, R : R + W]
            nc.vector.tensor_scalar_mul(yc, yrgb[:, :, 1, R : R + W], 0.299)
            nc.vector.tensor_scalar_mul(tmp0[:, :, :], yrgb[:, :, 2, R : R + W], 0.587)
            nc.vector.tensor_tensor(yc, yc, tmp0[:, :, :], Op.add)
            nc.vector.tensor_scalar_mul(tmp0[:, :, :], yrgb[:, :, 3, R : R + W], 0.114)
            nc.vector.tensor_tensor(yc, yc, tmp0[:, :, :], Op.add)
            nc.vector.tensor_copy(yrgb[:, :, :, 0:R], yrgb[:, :, :, W : W + R])
            nc.vector.tensor_copy(
                yrgb[:, :, :, W + R : W + 2 * R], yrgb[:, :, :, R : 2 * R]
            )

            # zero-shift term (w=1); den's +1 is added at the output stage
            nc.vector.tensor_copy(acc[:, :, :, :], yrgb[:, :, 1:4, R : R + W])
            ident = identd[:, :]
            pairs_total = sum(
                len(range(-R, R + 1)) if dy > 0 else len(range(1, R + 1))
                for dy in range(0, R + 1)
            )
            # den matmuls run in 512-element (one PSUM bank) units so that
            # start=True (which clears a whole bank) is exactly the first
            # write of each bank, and same-bank writes are WAW-ordered.
            n_den_banks = (C * W) // 512
            den_mm = [0]
            den_total = 2 * pairs_total * n_den_banks

            def den_accum(w_ap):
                dflat = denp[:, :, :].rearrange("p a b -> p (a b)")
                wflat = w_ap.rearrange("p a b -> p (a b)")
                for b in range(n_den_banks):
                    nc.tensor.matmul(
                        dflat[:, b * 512 : (b + 1) * 512],
                        ident,
                        wflat[:, b * 512 : (b + 1) * 512],
                        start=(den_mm[0] < n_den_banks),
                        stop=(den_mm[0] >= den_total - n_den_banks),
                        skip_group_check=True,
                    )
                    den_mm[0] += 1

            def box_stage(pool, tag, src, n_chunks, n_blocks, m_total, dst):
                """dst[:, b, m] = sum_k band-circulant matmul of src chunks."""
                for b in range(n_blocks):
                    ps = pool.tile([P, m_total], F32, tag=tag, name=tag)
                    mms = []
                    for t in range(n_chunks):
                        for g, off, run in _runs_mod(P * t - PR, BW, m_total):
                            mms.append((t, g, off, run))
                    for i, (t, g, off, run) in enumerate(mms):
                        nc.tensor.matmul(
                            ps[:, g : g + run],
                            src[:, t, b * P : (b + 1) * P],
                            band[:, off : off + run],
                            start=(i == 0),
                            stop=(i == len(mms) - 1),
                        )
                    if dst is not None:
                        nc.scalar.copy(dst[:, b, :], ps[:, :])
                    else:
                        yield b, ps

            # ---------------- shifts, grouped by dy ----------------
            for dy in range(0, R + 1):
                dxs = list(range(-R, R + 1)) if dy > 0 else list(range(1, R + 1))
                if dy == 0:
                    ysP = ysM = yrgb
                else:
                    # ysP[r] = yrgb[r-dy] ; ysM[r] = yrgb[r+dy] (rows circular)
                    ysP = rollp.tile([P, C, 4, WB], F32, tag="ysP", name="ysP")
                    ysM = rollp.tile([P, C, 4, WB], F32, tag="ysM", name="ysM")
                    nc.sync.dma_start(ysP[dy:P, :, :, :], yrgb[0 : P - dy, :, :, :])
                    if C > 1:
                        nc.sync.dma_start(
                            ysP[0:dy, 1:C, :, :], yrgb[P - dy : P, 0 : C - 1, :, :]
                        )
                    nc.sync.dma_start(
                        ysP[0:dy, 0, :, :], yrgb[P - dy : P, C - 1, :, :]
                    )
                    nc.sync.dma_start(ysM[0 : P - dy, :, :, :], yrgb[dy:P, :, :, :])
                    if C > 1:
                        nc.sync.dma_start(
                            ysM[P - dy : P, 0 : C - 1, :, :], yrgb[0:dy, 1:C, :, :]
                        )
                    nc.sync.dma_start(
                        ysM[P - dy : P, C - 1, :, :], yrgb[0:dy, 0, :, :]
                    )
                if dy > 0:
                    i12 = rollp.tile([P, 2 * P], F32, tag="i12", name="i12")
                    nc.sync.dma_start(
                        i12[:, :], id_in[:, 2 * (dy - 1) * P : 2 * dy * P]
                    )
                    i1 = i12[:, 0:P]
                    i2 = i12[:, P : 2 * P]

                for dx in dxs:
                    xf = slice(R - dx, R - dx + W)  # read at x - dx
                    xb = slice(R + dx, R + dx + W)  # read at x + dx

                    # D = (y - y(p-s))^2
                    dbuf = work.tile([P, C, W], F32, tag="d", name="dbuf", bufs=2)
                    nc.vector.tensor_tensor(
                        dbuf[:, :, :], yc, ysP[:, :, 0, xf], Op.subtract
                    )
                    nc.scalar.activation(dbuf[:, :, :], dbuf[:, :, :], AF.Square)

                    # box over rows (output transposed: [x, r])
                    t1s = work.tile([P, XB, H], F32, tag="t1s", name="t1s")
                    list(box_stage(psA, "t1ps", dbuf, C, XB, H, t1s))
                    # box over x (output back to [r, x]); dist = sqrt(B) straight
                    # from PSUM into x-haloed bm (B >= 0: all-positive sums)
                    bm = work.tile([P, C, WB], F32, tag="bm", name="bm")
                    for rb, ps in box_stage(psB, "bps", t1s, XB, C, W, None):
                        nc.scalar.activation(bm[:, rb, R : R + W], ps[:, :], AF.Sqrt)
                    bmc = bm[:, :, R : R + W]
                    # x halos of dist (for the backward x+dx read)
                    nc.scalar.copy(bm[:, :, 0:R], bm[:, :, W : W + R])
                    nc.scalar.copy(
                        bm[:, :, W + R : W + 2 * R], bm[:, :, R : 2 * R]
                    )

                    # w1 = exp(-dist/h); forward apply (ch 0,1 fused on DVE
                    # via a stride-0 broadcast of w1, ch 2 on GPSIMD)
                    w1 = work.tile([P, C, 1, W], F32, tag="w1", name="w1")
                    w1c = w1[:, :, 0, :]
                    nc.scalar.activation(w1c, bmc, AF.Exp, scale=nih[:, :])
                    u2 = work.tile([P, C, 2, W], F32, tag="u2", name="u2")
                    ug = work.tile([P, C, W], F32, tag="ug", name="ug")
                    nc.vector.tensor_tensor(
                        u2[:, :, :, :],
                        ysP[:, :, 1:3, xf],
                        w1[:, :, :, :].broadcast_to([P, C, 2, W]),
                        Op.mult,
                    )
                    nc.gpsimd.tensor_tensor(
                        ug[:, :, :], ysP[:, :, 3, xf], w1c, Op.mult
                    )
                    nc.vector.tensor_tensor(
                        acc[:, :, 0:2, :], acc[:, :, 0:2, :], u2[:, :, :, :], Op.add
                    )
                    nc.gpsimd.tensor_tensor(
                        acc[:, :, 2, :], acc[:, :, 2, :], ug[:, :, :], Op.add
                    )
                    den_accum(w1c)

                    # w2 = exp(-dist(p+s)/h)
                    w2 = work.tile([P, C, 1, W], F32, tag="w2", name="w2", bufs=2)
                    w2c = w2[:, :, 0, :]
                    if dy == 0:
                        nc.scalar.activation(
                            w2c, bm[:, :, xb], AF.Exp, scale=nih[:, :]
                        )
                    else:
                        # rows shifted by +dy on the PE: out[m] = dist[m+dy]
                        pss = []
                        for c in range(C):
                            ps = psD.tile([P, W], F32, tag="d2", name="d2")
                            nc.tensor.matmul(
                                ps[:, :], i1, bm[:, c, xb], start=True, stop=False
                            )
                            pss.append(ps)
                        for c in range(C):
                            nc.tensor.matmul(
                                pss[c][:, :],
                                i2,
                                bm[:, (c + 1) % C, xb],
                                start=False,
                                stop=True,
                            )
                        for c in range(C):
                            nc.scalar.activation(
                                w2[:, c, 0, :], pss[c][:, :], AF.Exp, scale=nih[:, :]
                            )

                    # backward apply: num += rgb(p+s)*w2 ; den += w2
                    # (ch 0,1 fused on DVE, ch 2 on GPSIMD, den on PE)
                    nc.vector.tensor_tensor(
                        u2[:, :, :, :],
                        ysM[:, :, 1:3, xb],
                        w2[:, :, :, :].broadcast_to([P, C, 2, W]),
                        Op.mult,
                    )
                    nc.gpsimd.tensor_tensor(
                        ug[:, :, :], ysM[:, :, 3, xb], w2c, Op.mult
                    )
                    nc.vector.tensor_tensor(
                        acc[:, :, 0:2, :], acc[:, :, 0:2, :], u2[:, :, :, :], Op.add
                    )
                    nc.gpsimd.tensor_tensor(
                        acc[:, :, 2, :], acc[:, :, 2, :], ug[:, :, :], Op.add
                    )
                    den_accum(w2c)

            # ---------------- output ----------------
            rden = work.tile([P, C, W], F32, tag="d", name="rden", bufs=2)
            # den = denp + 1 (zero-shift term)
            nc.vector.tensor_scalar_add(rden[:, :, :], denp[:, :, :], 1.0)
            nc.vector.reciprocal(rden[:, :, :], rden[:, :, :])
            for ch in range(3):
                nc.vector.tensor_tensor(
                    acc[:, :, ch, :], acc[:, :, ch, :], rden[:, :, :], Op.mult
                )
            nc.vector.tensor_scalar(
                acc[:, :, :, :], acc[:, :, :, :], 0.0, 1.0, Op.max, Op.min
            )
            for ch in range(3):
                out_dst = out_dram.ap()[ch].rearrange("(c p) x -> p c x", p=P)
                nc.sync.dma_start(out_dst, acc[:, :, ch, :])

    nc.compile()
    return nc


def _band_matrix():
    bw = P + 2 * PR
    i = np.arange(P)[:, None]
    j = np.arange(bw)[None, :]
    return (((j - i) >= 0) & ((j - i) <= 2 * PR)).astype(np.float32)


def _ident_matrices(R):
    """Packed shifted identities [P, (2*R+1)*P]: for dy in 1..R, I1
    (out[m]=in[m+dy] within chunk) then I2 (wrap rows from the next chunk);
    the last block is the plain identity."""
    out = np.zeros((P, (2 * R + 1) * P), np.float32)
    for dy in range(1, R + 1):
        i1 = np.zeros((P, P), np.float32)
        i2 = np.zeros((P, P), np.float32)
        for m in range(P - dy):
            i1[m + dy, m] = 1.0
        for m in range(P - dy, P):
            i2[m + dy - P, m] = 1.0
        out[:, 2 * (dy - 1) * P : (2 * dy - 1) * P] = i1
        out[:, (2 * dy - 1) * P : 2 * dy * P] = i2
    out[:, 2 * R * P : (2 * R + 1) * P] = np.eye(P, dtype=np.float32)
    return out


def get_nc(H=512, W=512, R=10, n_cores=8):
    key = (H, W, R, n_cores)
    if key not in _CACHE:
        _CACHE[key] = _build(H, W, R, n_cores)
    return _CACHE[key]


def run(rgb, h, H, W, R):
    """rgb [B,3,H,W], h [1] -> [B,3,H,W]; B must equal n_cores used."""
    from concourse.bass_utils import run_bass_kernel_spmd

    B = rgb.shape[0]
    nc = get_nc(H, W, R, B)
    band = _band_matrix()
    idents = _ident_matrices(R)
    hv = np.asarray(h, np.float32).reshape(1, 1)
    in_maps = [
        {
            "rgb": np.ascontiguousarray(rgb[i], np.float32),
            "h": hv,
            "band": band,
            "idents": idents,
        }
        for i in range(B)
    ]
    res = run_bass_kernel_spmd(nc, in_maps, list(range(B)))
    return np.stack([res.results[i]["out"] for i in range(B)], axis=0)


def kernel(rgb, h):
    rgb = np.asarray(rgb, np.float32)
    out = run(rgb, np.asarray(h, np.float32), 512, 512, 10)
    return out.astype(np.float32)


# revision 33
# speedup vs baseline: 469.6901x; 1.1617x over previous
"""Non-Local Means (gray-weighted) Bass kernel for Trainium2.

Contract: kernel(rgb, h) with rgb [8,3,512,512] f32, h [1] f32 -> [8,3,512,512] f32.
Data-parallel over batch: one image per NeuronCore (8 cores).

Algorithm (matches reference.py):
  y = luminance(clip(rgb,0,1)); for each shift s in [-R,R]^2:
    dist_s = sqrt(relu(box7((y - roll(y,s))^2)))   (circular boundary)
    w_s = exp(-dist_s/(relu(h)+eps))
    num += roll(rgb,s)*w_s ; den += w_s
  out = clip(num/den, 0, 1)

Mapping per core:
  - Symmetric pairs: dist_{-s}(p) = dist_s(p+s). Each pair (s,-s) computes one
    dist plane; the -s side uses dist read at +s (rows shifted on the
    TensorEngine via shifted-identity matmuls, x shifted via haloed AP reads)
    and a second exp.
  - Row-shifted copies of the [y,rgb] block are grouped by dy (one +dy and one
    -dy block per group) via SBUF->SBUF DMA; dx handled by x-offset reads
    against +-R x-halos.
  - 7x7 box = two banded-circulant matmul stages on the TensorEngine with the
    image data as the stationary operand; each stage transposes, so two
    stages land back in the original layout.
  - sqrt/exp on ScalarE; elementwise + accumulation on VectorE.
"""

import sys

sys.path.insert(0, "/opt/trn_rl_repo")

import numpy as np

EPS = 1e-8
PR = 3  # patch radius (7x7 box)
P = 128  # SBUF partitions

_CACHE = {}


def _runs_mod(start, length, m):
    """Split indices [(start+j) % m for j in range(length)] into contiguous
    runs; yields (out_start, window_offset, run_len)."""
    out = []
    j = 0
    while j < length:
        g = (start + j) % m
        run = min(length - j, m - g)
        out.append((g, j, run))
        j += run
    return out


def _build(H, W, R, n_cores):
    import concourse.bacc as bacc
    import concourse.mybir as mybir
    import concourse.tile as tile
    from concourse.mybir import ActivationFunctionType as AF
    from concourse.mybir import AluOpType as Op

    F32 = mybir.dt.float32
    C = H // P  # row chunks
    XB = W // P  # x blocks
    WB = W + 2 * R  # x-haloed width
    BW = P + 2 * PR  # band window width

    nc = bacc.Bacc(None, target_bir_lowering=False, debug=False)

    rgb_in = nc.dram_tensor("rgb", [3, H, W], F32, kind="ExternalInput")
    h_in = nc.dram_tensor("h", [1, 1], F32, kind="ExternalInput")
    band_in = nc.dram_tensor("band", [P, BW], F32, kind="ExternalInput")
    id_in = nc.dram_tensor("idents", [P, (2 * R + 1) * P], F32, kind="ExternalInput")
    out_dram = nc.dram_tensor("out", [3, H, W], F32, kind="ExternalOutput")

    with tile.TileContext(nc) as tc:
        with (
            tc.tile_pool(name="res", bufs=1) as res,
            tc.tile_pool(name="roll", bufs=1) as rollp,
            tc.tile_pool(name="work", bufs=1) as work,
            tc.tile_pool(name="psA", bufs=2, space="PSUM") as psA,
            tc.tile_pool(name="psB", bufs=1, space="PSUM") as psB,
            tc.tile_pool(name="psD", bufs=1, space="PSUM") as psD,
            tc.tile_pool(name="psDen", bufs=1, space="PSUM") as psDen,
        ):
            # ---------------- setup ----------------
            yrgb = res.tile([P, C, 4, WB], F32)  # plane 0=y, 1..3=rgb
            acc = res.tile([P, C, 3, W], F32)
            denp = psDen.tile([P, C, W], F32)  # den - 1, accumulated on the PE
            band = res.tile([P, BW], F32)
            identd = res.tile([P, P], F32)
            h_sb = res.tile([1, 1], F32)
            nih1 = res.tile([1, 1], F32)
            nih = res.tile([P, 1], F32)  # -1/(relu(h)+eps) on all partitions

            nc.sync.dma_start(band[:, :], band_in[:, :])
            nc.sync.dma_start(identd[:, :], id_in[:, 2 * R * P : (2 * R + 1) * P])
            nc.sync.dma_start(h_sb[:, :], h_in[:, :])
            nc.scalar.activation(h_sb[:, :], h_sb[:, :], AF.Relu)
            nc.vector.tensor_scalar_add(h_sb[:, :], h_sb[:, :], EPS)
            nc.vector.reciprocal(nih1[:, :], h_sb[:, :])
            nc.vector.tensor_scalar_mul(nih1[:, :], nih1[:, :], -1.0)
            nc.gpsimd.partition_broadcast(nih[:, :], nih1[:, :])

            for ch in range(3):
                rgb_src = rgb_in.ap()[ch].rearrange("(c p) x -> p c x", p=P)
                nc.sync.dma_start(yrgb[:, :, 1 + ch, R : R + W], rgb_src)
            nc.vector.tensor_scalar(
                yrgb[:, :, 1:4, R : R + W],
                yrgb[:, :, 1:4, R : R + W],
                0.0,
                1.0,
                Op.max,
                Op.min,
            )
            tmp0 = work.tile([P, C, W], F32, tag="d")
            yc = yrgb[:, :, 0, R : R + W]
            nc.vector.tensor_scalar_mul(yc, yrgb[:, :, 1, R : R + W], 0.299)
            nc.vector.tensor_scalar_mul(tmp0[:, :, :], yrgb[:, :, 2, R : R + W], 0.587)
            nc.vector.tensor_tensor(yc, yc, tmp0[:, :, :], Op.add)
            nc.vector.tensor_scalar_mul(tmp0[:, :, :], yrgb[:, :, 3, R : R + W], 0.114)
            nc.vector.tensor_tensor(yc, yc, tmp0[:, :, :], Op.add)
            nc.vector.tensor_copy(yrgb[:, :, :, 0:R], yrgb[:, :, :, W : W + R])
            nc.vector.tensor_copy(
                yrgb[:, :, :, W + R : W + 2 * R], yrgb[:, :, :, R : 2 * R]
            )

            # zero-shift term (w=1); den's +1 is added at the output stage
            nc.vector.tensor_copy(acc[:, :, :, :], yrgb[:, :, 1:4, R : R + W])
            ident = identd[:, :]
            pairs_total = sum(
                len(range(-R, R + 1)) if dy > 0 else len(range(1, R + 1))
                for dy in range(0, R + 1)
            )
            # den matmuls run in 512-element (one PSUM bank) units so that
            # start=True (which clears a whole bank) is exactly the first
            # write of each bank, and same-bank writes are WAW-ordered.
            n_den_banks = (C * W) // 512
            den_mm = [0]
            den_total = 2 * pairs_total * n_den_banks

            def den_accum(w_ap):
                dflat = denp[:, :, :].rearrange("p a b -> p (a b)")
                wflat = w_ap.rearrange("p a b -> p (a b)")
                for b in range(n_den_banks):
                    nc.tensor.matmul(
                        dflat[:, b * 512 : (b + 1) * 512],
                        ident,
                        wflat[:, b * 512 : (b + 1) * 512],
                        start=(den_mm[0] < n_den_banks),
                        stop=(den_mm[0] >= den_total - n_den_banks),
                        skip_group_check=True,
                    )
                    den_mm[0] += 1

            def box_stage(pool, tag, src, n_chunks, n_blocks, m_total, dst):
                """dst[:, b, m] = sum_k band-circulant matmul of src chunks."""
                for b in range(n_blocks):
                    ps = pool.tile([P, m_total], F32, tag=tag, name=tag)
                    mms = []
                    for t in range(n_chunks):
                        for g, off, run in _runs_mod(P * t - PR, BW, m_total):
                            mms.append((t, g, off, run))
                    for i, (t, g, off, run) in enumerate(mms):
                        nc.tensor.matmul(
                            ps[:, g : g + run],
                            src[:, t, b * P : (b + 1) * P],
                            band[:, off : off + run],
                            start=(i == 0),
                            stop=(i == len(mms) - 1),
                        )
                    if dst is not None:
                        nc.scalar.copy(dst[:, b, :], ps[:, :])
                    else:
                        yield b, ps

            # ---------------- shifts, grouped by dy ----------------
            for dy in range(0, R + 1):
                dxs = list(range(-R, R + 1)) if dy > 0 else list(range(1, R + 1))
                if dy == 0:
                    ysP = ysM = yrgb
                else:
                    # ysP[r] = yrgb[r-dy] ; ysM[r] = yrgb[r+dy] (rows circular)
                    ysP = rollp.tile([P, C, 4, WB], F32, tag="ysP", name="ysP")
                    ysM = rollp.tile([P, C, 4, WB], F32, tag="ysM", name="ysM")
                    nc.sync.dma_start(ysP[dy:P, :, :, :], yrgb[0 : P - dy, :, :, :])
                    if C > 1:
                        nc.sync.dma_start(
                            ysP[0:dy, 1:C, :, :], yrgb[P - dy : P, 0 : C - 1, :, :]
                        )
                    nc.sync.dma_start(
                        ysP[0:dy, 0, :, :], yrgb[P - dy : P, C - 1, :, :]
                    )
                    nc.sync.dma_start(ysM[0 : P - dy, :, :, :], yrgb[dy:P, :, :, :])
                    if C > 1:
                        nc.sync.dma_start(
                            ysM[P - dy : P, 0 : C - 1, :, :], yrgb[0:dy, 1:C, :, :]
                        )
                    nc.sync.dma_start(
                        ysM[P - dy : P, C - 1, :, :], yrgb[0:dy, 0, :, :]
                    )
                if dy > 0:
                    i12 = rollp.tile([P, 2 * P], F32, tag="i12", name="i12")
                    nc.sync.dma_start(
                        i12[:, :], id_in[:, 2 * (dy - 1) * P : 2 * dy * P]
                    )
                    i1 = i12[:, 0:P]
                    i2 = i12[:, P : 2 * P]

                for dx in dxs:
                    xf = slice(R - dx, R - dx + W)  # read at x - dx
                    xb = slice(R + dx, R + dx + W)  # read at x + dx

                    # D = (y - y(p-s))^2
                    dbuf = work.tile([P, C, W], F32, tag="d", name="dbuf")
                    nc.vector.tensor_tensor(
                        dbuf[:, :, :], yc, ysP[:, :, 0, xf], Op.subtract
                    )
                    nc.scalar.activation(dbuf[:, :, :], dbuf[:, :, :], AF.Square)

                    # box over rows (output transposed: [x, r])
                    t1s = work.tile([P, XB, H], F32, tag="t1s", name="t1s", bufs=2)
                    list(box_stage(psA, "t1ps", dbuf, C, XB, H, t1s))
                    # box over x (output back to [r, x]); dist = sqrt(B) straight
                    # from PSUM into x-haloed bm (B >= 0: all-positive sums)
                    bm = work.tile([P, C, WB], F32, tag="bm", name="bm", bufs=2)
                    for rb, ps in box_stage(psB, "bps", t1s, XB, C, W, None):
                        nc.scalar.activation(bm[:, rb, R : R + W], ps[:, :], AF.Sqrt)
                    bmc = bm[:, :, R : R + W]
                    # x halos of dist (for the backward x+dx read)
                    nc.scalar.copy(bm[:, :, 0:R], bm[:, :, W : W + R])
                    nc.scalar.copy(
                        bm[:, :, W + R : W + 2 * R], bm[:, :, R : 2 * R]
                    )

                    # w1 = exp(-dist/h); forward apply (ch 0,1 fused on DVE
                    # via a stride-0 broadcast of w1, ch 2 on GPSIMD)
                    w1 = work.tile([P, C, 1, W], F32, tag="w1", name="w1")
                    w1c = w1[:, :, 0, :]
                    nc.scalar.activation(w1c, bmc, AF.Exp, scale=nih[:, :])
                    u2 = work.tile([P, C, 2, W], F32, tag="u2", name="u2")
                    ug = work.tile([P, C, W], F32, tag="ug", name="ug")
                    nc.vector.tensor_tensor(
                        u2[:, :, :, :],
                        ysP[:, :, 1:3, xf],
                        w1[:, :, :, :].broadcast_to([P, C, 2, W]),
                        Op.mult,
                    )
                    nc.gpsimd.tensor_tensor(
                        ug[:, :, :], ysP[:, :, 3, xf], w1c, Op.mult
                    )
                    nc.vector.tensor_tensor(
                        acc[:, :, 0:2, :], acc[:, :, 0:2, :], u2[:, :, :, :], Op.add
                    )
                    nc.gpsimd.tensor_tensor(
                        acc[:, :, 2, :], acc[:, :, 2, :], ug[:, :, :], Op.add
                    )
                    den_accum(w1c)

                    # w2 = exp(-dist(p+s)/h)
                    w2 = work.tile([P, C, 1, W], F32, tag="w2", name="w2")
                    w2c = w2[:, :, 0, :]
                    if dy == 0:
                        nc.scalar.activation(
                            w2c, bm[:, :, xb], AF.Exp, scale=nih[:, :]
                        )
                    else:
                        # rows shifted by +dy on the PE: out[m] = dist[m+dy]
                        pss = []
                        for c in range(C):
                            ps = psD.tile([P, W], F32, tag="d2", name="d2")
                            nc.tensor.matmul(
                                ps[:, :], i1, bm[:, c, xb], start=True, stop=False
                            )
                            pss.append(ps)
                        for c in range(C):
                            nc.tensor.matmul(
                                pss[c][:, :],
                                i2,
                                bm[:, (c + 1) % C, xb],
                                start=False,
                                stop=True,
                            )
                        for c in range(C):
                            nc.scalar.activation(
                                w2[:, c, 0, :], pss[c][:, :], AF.Exp, scale=nih[:, :]
                            )

                    # backward apply: num += rgb(p+s)*w2 ; den += w2
                    # (ch 0,1 fused on DVE, ch 2 on GPSIMD, den on PE)
                    nc.vector.tensor_tensor(
                        u2[:, :, :, :],
                        ysM[:, :, 1:3, xb],
                        w2[:, :, :, :].broadcast_to([P, C, 2, W]),
                        Op.mult,
                    )
                    nc.gpsimd.tensor_tensor(
                        ug[:, :, :], ysM[:, :, 3, xb], w2c, Op.mult
                    )
                    nc.vector.tensor_tensor(
                        acc[:, :, 0:2, :], acc[:, :, 0:2, :], u2[:, :, :, :], Op.add
                    )
                    nc.gpsimd.tensor_tensor(
                        acc[:, :, 2, :], acc[:, :, 2, :], ug[:, :, :], Op.add
                    )
                    den_accum(w2c)

            # ---------------- output ----------------
            rden = work.tile([P, C, W], F32, tag="d", name="rden")
            # den = denp + 1 (zero-shift term)
            nc.vector.tensor_scalar_add(rden[:, :, :], denp[:, :, :], 1.0)
            nc.vector.reciprocal(rden[:, :, :], rden[:, :, :])
            for ch in range(3):
                nc.vector.tensor_tensor(
                    acc[:, :, ch, :], acc[:, :, ch, :], rden[:, :, :], Op.mult
                )
            nc.vector.tensor_scalar(
                acc[:, :, :, :], acc[:, :, :, :], 0.0, 1.0, Op.max, Op.min
            )
            for ch in range(3):
                out_dst = out_dram.ap()[ch].rearrange("(c p) x -> p c x", p=P)
                nc.sync.dma_start(out_dst, acc[:, :, ch, :])

    nc.compile()
    return nc


def _band_matrix():
    bw = P + 2 * PR
    i = np.arange(P)[:, None]
    j = np.arange(bw)[None, :]
    return (((j - i) >= 0) & ((j - i) <= 2 * PR)).astype(np.float32)


def _ident_matrices(R):
    """Packed shifted identities [P, (2*R+1)*P]: for dy in 1..R, I1
    (out[m]=in[m+dy] within chunk) then I2 (wrap rows from the next chunk);
    the last block is the plain identity."""
    out = np.zeros((P, (2 * R + 1) * P), np.float32)
    for dy in range(1, R + 1):
        i1 = np.zeros((P, P), np.float32)
        i2 = np.zeros((P, P), np.float32)
        for m in range(P - dy):
            i1[m + dy, m] = 1.0
        for m in range(P - dy, P):
            i2[m + dy - P, m] = 1.0
        out[:, 2 * (dy - 1) * P : (2 * dy - 1) * P] = i1
        out[:, (2 * dy - 1) * P : 2 * dy * P] = i2
    out[:, 2 * R * P : (2 * R + 1) * P] = np.eye(P, dtype=np.float32)
    return out


def get_nc(H=512, W=512, R=10, n_cores=8):
    key = (H, W, R, n_cores)
    if key not in _CACHE:
        _CACHE[key] = _build(H, W, R, n_cores)
    return _CACHE[key]


def run(rgb, h, H, W, R):
    """rgb [B,3,H,W], h [1] -> [B,3,H,W]; B must equal n_cores used."""
    from concourse.bass_utils import run_bass_kernel_spmd

    B = rgb.shape[0]
    nc = get_nc(H, W, R, B)
    band = _band_matrix()
    idents = _ident_matrices(R)
    hv = np.asarray(h, np.float32).reshape(1, 1)
    in_maps = [
        {
            "rgb": np.ascontiguousarray(rgb[i], np.float32),
            "h": hv,
            "band": band,
            "idents": idents,
        }
        for i in range(B)
    ]
    res = run_bass_kernel_spmd(nc, in_maps, list(range(B)))
    return np.stack([res.results[i]["out"] for i in range(B)], axis=0)


def kernel(rgb, h):
    rgb = np.asarray(rgb, np.float32)
    out = run(rgb, np.asarray(h, np.float32), 512, 512, 10)
    return out.astype(np.float32)
